# revision 1
# baseline (speedup 1.0000x reference)
"""Trainium2 Bass kernel for nn_Decoder_17489106830107 (VMamba VSSBlock decoder).

Sharding: one (batch, scan-direction) pair per core (B=2 x K=4 = 8 cores).
The host pre-permutes each core's inputs into that core's scan coordinate
order (transpose / 180-rotation of the image), so all 8 cores run ONE
identical SPMD program for launch 1 (conv1x1 -> LN -> in_proj -> depthwise
conv -> x_proj/dt -> 16 hardware linear-recurrence scans on the DVE).
The host then scatter-adds the 4 directional outputs per batch and an
8-way token-parallel launch 2 does the merge epilogue (out_norm, gating,
out_proj, MLP).
"""
import numpy as np
from contextlib import ExitStack

import concourse.bacc as bacc
import concourse.bass as bass
import concourse.mybir as mybir
import concourse.tile as tile
from concourse.bass_utils import run_bass_kernel_spmd
import ml_dtypes

F32 = mybir.dt.float32
F32R = mybir.dt.float32r
BF16 = mybir.dt.bfloat16
AF = mybir.ActivationFunctionType
OP = mybir.AluOpType

B, C, H, W = 2, 256, 64, 64
D = 64
Di = 128
N = 16
R = 4
K = 4
L = H * W          # 4096
LC = 1024          # scan chunk
NCH = L // LC
EPS = 1e-5
T2 = 1024          # launch-2 token slice per core


# ---------------------------------------------------------------- host prep

def _perms():
    ar = np.arange(L)
    p1 = (ar % 64) * 64 + ar // 64
    return [ar, p1, ar[::-1].copy(), p1[::-1].copy()]


def _permute_kernel(w, k):
    if k == 0:
        return w
    if k == 1:
        return w.transpose(0, 2, 1)
    if k == 2:
        return w[:, ::-1, ::-1]
    return w.transpose(0, 2, 1)[:, ::-1, ::-1]


# ---------------------------------------------------------------- launch 1

def _r(ap):
    return ap.bitcast(F32R)


def build_launch1():
    nc = bacc.Bacc("TRN2", target_bir_lowering=False, debug=False,
                   num_devices=8)

    def inp(name, shape):
        return nc.dram_tensor(name, shape, F32, kind="ExternalInput")

    xin = inp("xin", [3 * C, L])
    convT = inp("convT", [3 * C, D])
    conv_b = inp("conv_b", [D, 1])
    sel = inp("sel", [128, 2])
    ones1 = inp("ones1", [1, 128])
    Wp = nc.dram_tensor("Wp", [D, 2 * Di], BF16,
                        kind="ExternalInput")
    negq = inp("negq", [128, 2])
    bias_z = inp("bias_z", [128, 1])
    dwdiag = nc.dram_tensor("dwdiag", [9, 128, 128], BF16,
                            kind="ExternalInput")
    bias_dw = inp("bias_dw", [128, 1])
    xprojT = nc.dram_tensor("xprojT", [Di, R + 2 * N], BF16,
                            kind="ExternalInput")
    dtT = nc.dram_tensor("dtT", [R, Di], BF16,
                         kind="ExternalInput")
    dtb = inp("dtb", [Di, 1])
    A_in = inp("A_in", [Di, N])
    bsel = nc.dram_tensor("bsel", [R + 2 * N, N * 128], BF16,
                          kind="ExternalInput")
    csel = nc.dram_tensor("csel", [R + 2 * N, N * 128], BF16,
                          kind="ExternalInput")
    Ds_in = inp("Ds_in", [Di, 1])

    y_out = nc.dram_tensor("y_out", [Di, L], F32, kind="ExternalOutput")
    sz_out = nc.dram_tensor("sz_out", [Di, L], F32, kind="ExternalOutput")
    x_out = nc.dram_tensor("x_out", [D, L], F32, kind="ExternalOutput")

    with tile.TileContext(nc) as tc, ExitStack() as ctx:
        cpool = ctx.enter_context(tc.tile_pool(name="consts", bufs=1))
        main = ctx.enter_context(tc.tile_pool(name="main", bufs=1))

        # ---- const loads
        convT_sb = cpool.tile([128, 6, D], F32, tag="convT")
        nc.sync.dma_start(convT_sb[:], convT[:].rearrange("(c p) m -> p c m", p=128))
        conv_b_sb = cpool.tile([D, 1], F32, tag="convb")
        nc.sync.dma_start(conv_b_sb[:], conv_b[:])
        sel_sb = cpool.tile([128, 2], F32, tag="sel")
        nc.sync.dma_start(sel_sb[:], sel[:])
        ones1_sb = cpool.tile([1, 128], F32, tag="ones1")
        nc.sync.dma_start(ones1_sb[:], ones1[:])
        Wp_sb = cpool.tile([D, 2 * Di], BF16, tag="Wp")
        nc.sync.dma_start(Wp_sb[:], Wp[:])
        negq_sb = cpool.tile([128, 2], F32, tag="negq")
        nc.sync.dma_start(negq_sb[:], negq[:])
        bias_z_sb = cpool.tile([128, 1], F32, tag="biasz")
        nc.sync.dma_start(bias_z_sb[:], bias_z[:])
        dwdiag_sb = cpool.tile([128, 9, 128], BF16, tag="dwdiag")
        nc.sync.dma_start(dwdiag_sb[:], dwdiag[:].rearrange("t p f -> p t f"))
        bias_dw_sb = cpool.tile([128, 1], F32, tag="biasdw")
        nc.sync.dma_start(bias_dw_sb[:], bias_dw[:])
        xprojT_sb = cpool.tile([Di, R + 2 * N], BF16, tag="xprojT")
        nc.sync.dma_start(xprojT_sb[:], xprojT[:])
        dtT_sb = cpool.tile([R, Di], BF16, tag="dtT")
        nc.sync.dma_start(dtT_sb[:], dtT[:])
        dtb_sb = cpool.tile([Di, 1], F32, tag="dtb")
        nc.sync.dma_start(dtb_sb[:], dtb[:])
        A_sb = cpool.tile([Di, N], F32, tag="A")
        nc.sync.dma_start(A_sb[:], A_in[:])
        Ds_sb = cpool.tile([Di, 1], F32, tag="Ds")
        nc.sync.dma_start(Ds_sb[:], Ds_in[:])
        eps_sb = cpool.tile([128, 1], F32, tag="eps")
        nc.vector.memset(eps_sb[:], EPS)
        bsel_sb = cpool.tile([R + 2 * N, N * 128], BF16, tag="bsel")
        nc.sync.dma_start(bsel_sb[:], bsel[:])
        csel_sb = cpool.tile([R + 2 * N, N * 128], BF16, tag="csel")
        nc.sync.dma_start(csel_sb[:], csel[:])

        # ---- persistent activations
        xc_sb = main.tile([Di, L], BF16, tag="xc")
        xdbl_bf = main.tile([R + 2 * N, L], BF16, tag="xdblbf")
        delta_sb = main.tile([Di, L], F32, tag="delta")
        du_sb = main.tile([Di, L], F32, tag="du")
        carry_sb = main.tile([Di, N], F32, tag="carry")

        with tc.tile_pool(name="imgp", bufs=1) as imgp:
            img = imgp.tile([Di, 66 * 66], BF16, tag="img")
            nc.gpsimd.memset(img[:], 0.0)
            img3 = img[:].rearrange("p (h w) -> p h w", h=66)

            with tc.tile_pool(name="p1", bufs=1) as p1, \
                 tc.tile_pool(name="p1x", bufs=3) as p1x:
                lnin = p1.tile([128, L], F32, tag="lnin")

                # conv1x1: psum[f] accumulates over 6 channel chunks
                with tc.tile_pool(name="ps_conv", bufs=1,
                                  space="PSUM") as ps_conv:
                    cps = [ps_conv.tile([D, 512], F32, tag=f"cps{f}",
                                        name=f"cps{f}")
                           for f in range(8)]
                    for c in range(6):
                        xin_c = p1x.tile([128, L], F32, tag="xin")
                        nc.sync.dma_start(xin_c[:],
                                          xin[:][c * 128:(c + 1) * 128, :])
                        for f in range(8):
                            nc.tensor.matmul(cps[f][:], convT_sb[:, c, :],
                                             xin_c[:, f * 512:(f + 1) * 512],
                                             start=(c == 0), stop=(c == 5))
                    for f in range(8):
                        nc.scalar.activation(lnin[0:D, f * 512:(f + 1) * 512],
                                             cps[f][:], AF.Identity,
                                             bias=conv_b_sb[:, 0:1])
                nc.sync.dma_start(x_out[:], lnin[0:D, :])
                lnin_bf = p1.tile([D, L], BF16, tag="lninbf")
                nc.scalar.copy(lnin_bf[:], lnin[0:D, :])

                # ---- LN1 stats, fully per-half so downstream starts early
                mu_b = p1.tile([128, L], F32, tag="mu_b")
                rs_b = p1.tile([128, L], F32, tag="rs_b")
                with tc.tile_pool(name="ps_st", bufs=1, space="PSUM") as ps_st:
                    for hh in range(2):
                        hsl = slice(hh * 2048, (hh + 1) * 2048)
                        nc.scalar.square(lnin[D:2 * D, hsl], lnin[0:D, hsl])
                        st0 = ps_st.tile([1, L // 2], F32, tag="st0",
                                         name="st0")
                        st1 = ps_st.tile([1, L // 2], F32, tag="st1",
                                         name="st1")
                        for f in range(4):
                            fsl = slice(hh * 2048 + f * 512,
                                        hh * 2048 + (f + 1) * 512)
                            psl = slice(f * 512, (f + 1) * 512)
                            nc.tensor.matmul(st0[:, psl], sel_sb[:, 0:1],
                                             lnin[:, fsl], start=True,
                                             stop=True)
                            nc.tensor.matmul(st1[:, psl], sel_sb[:, 1:2],
                                             lnin[:, fsl], start=True,
                                             stop=True)
                        nc.scalar.copy(mu_b[0:1, hsl], st0[:])
                        nc.scalar.copy(rs_b[0:1, hsl], st1[:])
                        s0r = p1.tile([128, 16], F32, tag="s0r", bufs=2)
                        s1r = p1.tile([128, 16], F32, tag="s1r", bufs=2)
                        nc.sync.dma_start(s0r[:], mu_b[0:1, hsl])
                        nc.sync.dma_start(s1r[:], rs_b[0:1, hsl])
                        m_r = p1.tile([128, 16], F32, tag="m_r", bufs=2)
                        nc.scalar.mul(m_r[:], s0r[:], 1.0 / D)
                        msq = p1.tile([128, 16], F32, tag="msq", bufs=2)
                        nc.scalar.square(msq[:], m_r[:])
                        v_r = p1.tile([128, 16], F32, tag="v_r", bufs=2)
                        nc.vector.scalar_tensor_tensor(v_r[:], s1r[:], 1.0 / D,
                                                       msq[:], OP.mult,
                                                       OP.subtract)
                        sd_r = p1.tile([128, 16], F32, tag="sd_r", bufs=2)
                        nc.scalar.activation(sd_r[:], v_r[:], AF.Sqrt,
                                             bias=eps_sb[:, 0:1])
                        rs_r = p1.tile([128, 16], F32, tag="rs_r", bufs=2)
                        nc.vector.reciprocal(rs_r[:], sd_r[:])
                        nc.sync.dma_start(mu_b[0:1, hsl], m_r[:])
                        nc.sync.dma_start(rs_b[0:1, hsl], rs_r[:])
                        nc.gpsimd.partition_broadcast(mu_b[:, hsl],
                                                      mu_b[0:1, hsl])
                        nc.gpsimd.partition_broadcast(rs_b[:, hsl],
                                                      rs_b[0:1, hsl])

                # ---- in_proj + LN fixup
                with tc.tile_pool(name="ps_ip", bufs=4, space="PSUM") as ps_ip, \
                     tc.tile_pool(name="fix", bufs=3) as fix:
                    for mc in range(2):
                        for f in range(8):
                            fsl = slice(f * 512, (f + 1) * 512)
                            pp = ps_ip.tile([128, 512], F32, tag="ipps")
                            nc.tensor.matmul(pp[:],
                                             Wp_sb[:, mc * 128:(mc + 1) * 128],
                                             lnin_bf[:, fsl],
                                             start=True, stop=True)
                            t1 = fix.tile([128, 512], F32, tag="t1")
                            nc.vector.scalar_tensor_tensor(
                                t1[:], mu_b[:, fsl], negq_sb[:, mc:mc + 1],
                                pp[:], OP.mult, OP.add)
                            if mc == 0:
                                r0 = f * 8
                                nc.vector.tensor_tensor(
                                    img3[:, 1 + r0:1 + r0 + 8, 1:65],
                                    t1[:].rearrange("p (r w) -> p r w", r=8),
                                    rs_b[:, fsl].rearrange("p (r w) -> p r w",
                                                           r=8),
                                    OP.mult)
                            else:
                                t2 = fix.tile([128, 512], F32, tag="t2")
                                nc.vector.tensor_tensor(t2[:], t1[:],
                                                        rs_b[:, fsl], OP.mult)
                                szt = fix.tile([128, 512], F32, tag="szt")
                                nc.scalar.activation(szt[:], t2[:],
                                                     AF.Silu,
                                                     bias=bias_z_sb[:, 0:1])
                                nc.sync.dma_start(sz_out[:][:, fsl], szt[:])

            # ---- interleaved per-chunk pipeline: dwconv -> x_proj ->
            # delta -> scans, so chunk c's scans overlap chunk c+1's prep
            dts_sb = main.tile([R, L], BF16, tag="dts")
            with tc.tile_pool(name="ps_dw", bufs=1, space="PSUM") as ps_dw, \
                 tc.tile_pool(name="ps_xp", bufs=1, space="PSUM") as ps_xp, \
                 tc.tile_pool(name="ps_dt", bufs=1, space="PSUM") as ps_dt, \
                 tc.tile_pool(name="ps_bb", bufs=1, space="PSUM") as ps_bb, \
                 tc.tile_pool(name="nl", bufs=4) as nl, \
                 tc.tile_pool(name="yp", bufs=2) as yp:
            
                for c in range(NCH):
                    csl = slice(c * LC, (c + 1) * LC)
                    for f in range(2 * c, 2 * c + 2):
                        fsl = slice(f * 512, (f + 1) * 512)
                        r0 = f * 8
                        dps = ps_dw.tile([128, 512], F32, tag="dwps")
                        for t in range(9):
                            di_, dj = t // 3, t % 3
                            nc.tensor.matmul(
                                dps[:], dwdiag_sb[:, t, :],
                                img3[:, r0 + di_:r0 + di_ + 8, dj:dj + 64],
                                start=(t == 0), stop=(t == 8))
                        nc.scalar.activation(xc_sb[:, fsl], dps[:], AF.Silu,
                                             bias=bias_dw_sb[:, 0:1])
                        xps = ps_xp.tile([R + 2 * N, 512], F32, tag="xpps")
                        nc.tensor.matmul(xps[:], xprojT_sb[:], xc_sb[:, fsl],
                                         start=True, stop=True)
                        nc.scalar.copy(xdbl_bf[:, fsl], xps[:])
                        nc.scalar.copy(dts_sb[:, fsl], xps[0:R, :])
                        dtps = ps_dt.tile([Di, 512], F32, tag="dtps")
                        nc.tensor.matmul(dtps[:], dtT_sb[:], dts_sb[:, fsl],
                                         start=True, stop=True)
                        nc.scalar.activation(delta_sb[:, fsl], dtps[:],
                                             AF.Sigmoid,
                                             bias=dtb_sb[:, 0:1], scale=-1.0)
                    nc.scalar.activation(delta_sb[:, csl], delta_sb[:, csl],
                                         AF.Ln)
                    nc.vector.scalar_tensor_tensor(du_sb[:, csl],
                                                   delta_sb[:, csl],
                                                   -1.0, xc_sb[:, csl],
                                                   OP.mult, OP.mult)
                    y_acc = yp.tile([Di, LC], F32, tag="yacc")
                    for n in range(N):
                        bb = ps_bb.tile([128, LC], F32, tag="bb")
                        for j in range(LC // 512):
                            nc.tensor.matmul(
                                bb[:, j * 512:(j + 1) * 512],
                                bsel_sb[:, n * 128:(n + 1) * 128],
                                xdbl_bf[:, c * LC + j * 512:
                                        c * LC + (j + 1) * 512],
                                start=True, stop=True)
                        cb = ps_bb.tile([128, LC], F32, tag="cb")
                        for j in range(LC // 512):
                            nc.tensor.matmul(
                                cb[:, j * 512:(j + 1) * 512],
                                csel_sb[:, n * 128:(n + 1) * 128],
                                xdbl_bf[:, c * LC + j * 512:
                                        c * LC + (j + 1) * 512],
                                start=True, stop=True)
                        da = nl.tile([Di, LC], F32, tag="da")
                        nc.scalar.activation(da[:], delta_sb[:, csl], AF.Exp,
                                             scale=A_sb[:, n:n + 1])
                        dbu = nl.tile([Di, LC], F32, tag="dbu")
                        nc.vector.tensor_tensor(dbu[:], du_sb[:, csl], bb[:],
                                                OP.mult)
                        h = nl.tile([Di, LC], F32, tag="h")
                        nc.vector.tensor_tensor_scan(
                            h[:], da[:], dbu[:],
                            0.0 if c == 0 else carry_sb[:, n:n + 1],
                            OP.mult, OP.add)
                        if c < NCH - 1:
                            nc.scalar.copy(carry_sb[:, n:n + 1],
                                           h[:, LC - 1:LC])
                        if n == 0:
                            nc.vector.tensor_tensor(y_acc[:], h[:], cb[:],
                                                    OP.mult)
                        else:
                            tmp = nl.tile([Di, LC], F32, tag="tmp")
                            nc.vector.tensor_tensor(tmp[:], h[:], cb[:],
                                                    OP.mult)
                            nc.gpsimd.tensor_tensor(y_acc[:], y_acc[:],
                                                    tmp[:], OP.add)
                    y_f = yp.tile([Di, LC], F32, tag="yout")
                    nc.vector.scalar_tensor_tensor(y_f[:], xc_sb[:, csl],
                                                   Ds_sb[:, 0:1], y_acc[:],
                                                   OP.mult, OP.add)
                    nc.sync.dma_start(y_out[:][:, csl], y_f[:])

    nc.compile()
    return nc


def _finish(nc):
    return nc


# ---------------------------------------------------------------- launch 2

def build_launch2():
    nc = bacc.Bacc("TRN2", target_bir_lowering=False, debug=False,
                   num_devices=8)

    def inp(name, shape):
        return nc.dram_tensor(name, shape, F32, kind="ExternalInput")

    y_in = nc.dram_tensor("y_in", [Di, T2], BF16, kind="ExternalInput")
    sz_in = nc.dram_tensor("sz_in", [Di, T2], BF16, kind="ExternalInput")
    x_in = inp("x_in", [D, T2])
    ones128 = inp("ones128", [128, 1])
    onorm_g = inp("onorm_g", [Di, 1])
    onorm_b = inp("onorm_b", [Di, 1])
    oproj = inp("oproj", [Di, D])
    fc1p = inp("fc1p", [D, 2 * Di])
    bias1 = inp("bias1", [128, 2])
    fc2w = inp("fc2w", [2 * Di, D])
    fc2b = inp("fc2b", [D, 1])
    out = nc.dram_tensor("out", [D, T2], F32, kind="ExternalOutput")

    with tile.TileContext(nc) as tc, ExitStack() as ctx:
        po = ctx.enter_context(tc.tile_pool(name="main", bufs=1))
        ps = ctx.enter_context(tc.tile_pool(name="psum", bufs=1, space="PSUM"))

        y_sb = po.tile([Di, T2], BF16, tag="y")
        nc.sync.dma_start(y_sb[:], y_in[:])
        sz_sb = po.tile([Di, T2], BF16, tag="sz")
        nc.sync.dma_start(sz_sb[:], sz_in[:])
        x_sb = po.tile([D, T2], F32, tag="x")
        nc.sync.dma_start(x_sb[:], x_in[:])
        ones_sb = po.tile([128, 1], F32, tag="ones")
        nc.sync.dma_start(ones_sb[:], ones128[:])
        ones_bf = po.tile([128, 1], BF16, tag="onesbf")
        nc.vector.memset(ones_bf[:], 1.0)
        og_sb = po.tile([Di, 1], F32, tag="og")
        nc.sync.dma_start(og_sb[:], onorm_g[:])
        ob_sb = po.tile([Di, 1], F32, tag="ob")
        nc.sync.dma_start(ob_sb[:], onorm_b[:])
        op_sb = po.tile([Di, D], F32, tag="oproj")
        nc.sync.dma_start(op_sb[:], oproj[:])
        fc1_sb = po.tile([D, 2 * Di], F32, tag="fc1")
        nc.sync.dma_start(fc1_sb[:], fc1p[:])
        b1_sb = po.tile([128, 2], F32, tag="b1")
        nc.sync.dma_start(b1_sb[:], bias1[:])
        fc2_sb = po.tile([128, 2, D], F32, tag="fc2")
        nc.sync.dma_start(fc2_sb[:], fc2w[:].rearrange("(c p) m -> p c m", p=128))
        fc2b_sb = po.tile([D, 1], F32, tag="fc2b")
        nc.sync.dma_start(fc2b_sb[:], fc2b[:])
        eps_sb = po.tile([128, 1], F32, tag="eps")
        nc.vector.memset(eps_sb[:], EPS)

        def pln(src, parts, tag, dt=F32, ones_t=None):
            """LayerNorm stats over the partition dim of src [parts, T2];
            returns broadcast (mu_b, rs_b) [parts, T2] tiles."""
            ones_t = ones_sb if ones_t is None else ones_t
            sq = po.tile([parts, T2], dt, tag=tag + "sq")
            nc.scalar.square(sq[:], src)
            st0_sb = po.tile([1, T2], F32, tag=tag + "st0sb")
            st1_sb = po.tile([1, T2], F32, tag=tag + "st1sb")
            with tc.tile_pool(name=tag + "ps_st", bufs=1,
                              space="PSUM") as ps_st:
                st0 = ps_st.tile([1, T2], F32, tag="st0")
                st1 = ps_st.tile([1, T2], F32, tag="st1")
                for f in range(T2 // 512):
                    fsl = slice(f * 512, (f + 1) * 512)
                    nc.tensor.matmul(st0[:, fsl], ones_t[0:parts, :],
                                     src[:, fsl], start=True, stop=True)
                    nc.tensor.matmul(st1[:, fsl], ones_t[0:parts, :],
                                     sq[:, fsl], start=True, stop=True)
                nc.scalar.copy(st0_sb[:], st0[:])
                nc.scalar.copy(st1_sb[:], st1[:])
            s0r = po.tile([128, T2 // 128], F32, tag=tag + "s0r")
            s1r = po.tile([128, T2 // 128], F32, tag=tag + "s1r")
            nc.sync.dma_start(s0r[:], st0_sb[:])
            nc.sync.dma_start(s1r[:], st1_sb[:])
            m_r = po.tile([128, T2 // 128], F32, tag=tag + "m")
            nc.scalar.mul(m_r[:], s0r[:], 1.0 / parts)
            msq = po.tile([128, T2 // 128], F32, tag=tag + "msq")
            nc.scalar.square(msq[:], m_r[:])
            v_r = po.tile([128, T2 // 128], F32, tag=tag + "v")
            nc.vector.scalar_tensor_tensor(v_r[:], s1r[:], 1.0 / parts,
                                           msq[:], OP.mult, OP.subtract)
            sd_r = po.tile([128, T2 // 128], F32, tag=tag + "sd")
            nc.scalar.activation(sd_r[:], v_r[:], AF.Sqrt, bias=eps_sb[:parts if False else 128, 0:1])
            rs_r = po.tile([128, T2 // 128], F32, tag=tag + "rs")
            nc.vector.reciprocal(rs_r[:], sd_r[:])
            mu1 = po.tile([1, T2], F32, tag=tag + "mu1")
            rs1 = po.tile([1, T2], F32, tag=tag + "rs1")
            nc.sync.dma_start(mu1[:], m_r[:])
            nc.sync.dma_start(rs1[:], rs_r[:])
            mu_b = po.tile([parts, T2], F32, tag=tag + "mub")
            rs_b = po.tile([parts, T2], F32, tag=tag + "rsb")
            nc.gpsimd.partition_broadcast(mu_b[:], mu1[:])
            nc.gpsimd.partition_broadcast(rs_b[:], rs1[:])
            return mu_b, rs_b

        # out_norm (over Di) + gate
        mu_b, rs_b = pln(y_sb[:], Di, "a", dt=BF16, ones_t=ones_bf)
        t1 = po.tile([Di, T2], F32, tag="t1")
        nc.vector.tensor_tensor(t1[:], y_sb[:], mu_b[:], OP.subtract)
        t2 = po.tile([Di, T2], F32, tag="t2")
        nc.vector.tensor_tensor(t2[:], t1[:], rs_b[:], OP.mult)
        t3 = po.tile([Di, T2], F32, tag="t3")
        nc.vector.tensor_scalar(t3[:], t2[:], og_sb[:, 0:1], ob_sb[:, 0:1],
                                OP.mult, OP.add)
        yg = po.tile([Di, T2], F32, tag="yg")
        nc.vector.tensor_tensor(yg[:], t3[:], sz_sb[:], OP.mult)

        # out_proj + residual ;  "mm" psum tag shared/serialized
        x2 = po.tile([D, T2], F32, tag="x2")
        opps = ps.tile([128, T2], F32, tag="mm")
        for f in range(T2 // 512):
            fsl = slice(f * 512, (f + 1) * 512)
            nc.tensor.matmul(opps[0:D, fsl], op_sb[:], yg[:, fsl],
                             start=True, stop=True)
        nc.vector.tensor_tensor(x2[:], opps[0:D, :], x_sb[:], OP.add)

        # LN2 (over D) -> fc1 -> gelu -> fc2 -> + residual
        mu2, rs2 = pln(x2[:], D, "b")
        h1 = po.tile([D, T2], F32, tag="h1")
        nc.vector.tensor_tensor(h1[:], x2[:], mu2[:], OP.subtract)
        hn = po.tile([D, T2], F32, tag="hn")
        nc.vector.tensor_tensor(hn[:], h1[:], rs2[:], OP.mult)

        g1 = po.tile([128, 2, T2], F32, tag="g1")
        for mc in range(2):
            fp = ps.tile([128, T2], F32, tag="mm")
            for f in range(T2 // 512):
                fsl = slice(f * 512, (f + 1) * 512)
                nc.tensor.matmul(fp[:, fsl],
                                 fc1_sb[:, mc * 128:(mc + 1) * 128],
                                 hn[:, fsl], start=True, stop=True)
            nc.scalar.activation(g1[:, mc, :], fp[:],
                                 AF.Gelu_apprx_tanh, bias=b1_sb[:, mc:mc + 1])
        f2 = ps.tile([128, T2], F32, tag="mm")
        for f in range(T2 // 512):
            fsl = slice(f * 512, (f + 1) * 512)
            for mc in range(2):
                nc.tensor.matmul(f2[0:D, fsl], fc2_sb[:, mc, :],
                                 g1[:, mc, fsl],
                                 start=(mc == 0), stop=(mc == 1))
        o_sb = po.tile([D, T2], F32, tag="o")
        nc.vector.scalar_tensor_tensor(o_sb[:], f2[0:D, :], fc2b_sb[:, 0:1],
                                       x2[:], OP.add, OP.add)
        nc.sync.dma_start(out[:], o_sb[:])

    nc.compile()
    return nc


# ---------------------------------------------------------------- host side

_CACHE = {}


def _get_programs():
    if "nc1" not in _CACHE:
        _CACHE["nc1"] = build_launch1()
        _CACHE["nc2"] = build_launch2()
    return _CACHE["nc1"], _CACHE["nc2"]


def _prep_inmaps(inputs):
    f32 = lambda a: np.ascontiguousarray(np.asarray(a), dtype=np.float32)
    conv_w = f32(inputs["conv_w"])
    conv_b = f32(inputs["conv_b"])
    ln1_g, ln1_b = f32(inputs["ln1_g"]), f32(inputs["ln1_b"])
    in_proj_w = f32(inputs["in_proj_w"])
    dw_w_all = f32(inputs["conv_dw_w"])[:, 0]
    dw_b = f32(inputs["conv_dw_b"])
    x_proj_w = f32(inputs["x_proj_w"])
    dt_proj_w = f32(inputs["dt_proj_w"])
    dt_proj_b = f32(inputs["dt_proj_b"])
    A = np.exp(f32(inputs["A_logs"])).reshape(K, Di, N).astype(np.float32)
    Ds = f32(inputs["Ds"]).reshape(K, Di)

    Wp = (ln1_g[:, None] * in_proj_w).astype(np.float32)        # [64, 256]
    Wp_bf = Wp.astype(ml_dtypes.bfloat16)
    q = Wp.sum(0)
    bias_full = (ln1_b @ in_proj_w).astype(np.float32)          # [256]
    negq = np.ascontiguousarray(np.stack([-q[:Di], -q[Di:]], 1), np.float32)
    sel = np.zeros((128, 2), np.float32)
    sel[:D, 0] = 1.0
    sel[D:, 1] = 1.0
    ones1 = np.ones((1, 128), np.float32)

    Ps = _perms()
    x123 = [np.concatenate([f32(inputs["x1"])[b], f32(inputs["x2"])[b],
                            f32(inputs["x3"])[b]], 0).reshape(3 * C, L)
            for b in range(B)]

    bsel_np = np.zeros((R + 2 * N, N * 128), ml_dtypes.bfloat16)
    csel_np = np.zeros((R + 2 * N, N * 128), ml_dtypes.bfloat16)
    for n in range(N):
        bsel_np[R + n, n * 128:(n + 1) * 128] = 1.0
        csel_np[R + N + n, n * 128:(n + 1) * 128] = 1.0
    shared = {
        "bsel": bsel_np, "csel": csel_np,
        "convT": np.ascontiguousarray(conv_w.T),
        "conv_b": conv_b.reshape(D, 1).copy(),
        "sel": sel, "ones1": ones1, "Wp": Wp_bf, "negq": negq,
        "bias_z": bias_full[Di:].reshape(Di, 1).copy(),
    }
    in_maps = []
    for core in range(8):
        b, k = core // 4, core % 4
        dw_w = _permute_kernel(dw_w_all, k)
        wsum = dw_w.sum((1, 2))
        dwdiag = np.zeros((9, 128, 128), ml_dtypes.bfloat16)
        for t in range(9):
            np.fill_diagonal(dwdiag[t], dw_w[:, t // 3, t % 3])
        in_maps.append({
            **shared,
            "xin": np.ascontiguousarray(x123[b][:, Ps[k]]),
            "dwdiag": dwdiag,
            "bias_dw": (dw_b + bias_full[:Di] * wsum).reshape(Di, 1)
                        .astype(np.float32),
            "xprojT": np.ascontiguousarray(x_proj_w[k].T.astype(ml_dtypes.bfloat16)),
            "dtT": np.ascontiguousarray(dt_proj_w[k].T.astype(ml_dtypes.bfloat16)),
            "dtb": (-dt_proj_b[k]).reshape(Di, 1).copy(),
            "A_in": np.ascontiguousarray(A[k]),
            "Ds_in": Ds[k].reshape(Di, 1).copy(),
        })
    return in_maps, Ps


def _prep_inmaps2(inputs, y_merged, sz_full, x_full):
    f32 = lambda a: np.ascontiguousarray(np.asarray(a), dtype=np.float32)
    ln2_g, ln2_b = f32(inputs["ln2_g"]), f32(inputs["ln2_b"])
    fc1_w, fc1_b = f32(inputs["fc1_w"]), f32(inputs["fc1_b"])
    fc1p = (ln2_g[:, None] * fc1_w).astype(np.float32)
    bias1 = (ln2_b @ fc1_w + fc1_b).astype(np.float32)
    shared = {
        "ones128": np.ones((128, 1), np.float32),
        "onorm_g": f32(inputs["out_norm_g"]).reshape(Di, 1).copy(),
        "onorm_b": f32(inputs["out_norm_b"]).reshape(Di, 1).copy(),
        "oproj": f32(inputs["out_proj_w"]),
        "fc1p": fc1p,
        "bias1": np.ascontiguousarray(np.stack([bias1[:128], bias1[128:]], 1),
                                      np.float32),
        "fc2w": f32(inputs["fc2_w"]),
        "fc2b": f32(inputs["fc2_b"]).reshape(D, 1).copy(),
    }
    in_maps = []
    for core in range(8):
        b, sl = core // 4, slice((core % 4) * T2, (core % 4 + 1) * T2)
        in_maps.append({
            **shared,
            "y_in": np.ascontiguousarray(
                y_merged[b][:, sl].astype(ml_dtypes.bfloat16)),
            "sz_in": np.ascontiguousarray(
                sz_full[b][:, sl].astype(ml_dtypes.bfloat16)),
            "x_in": np.ascontiguousarray(x_full[b][:, sl]),
        })
    return in_maps


def kernel(**inputs):
    nc1, nc2 = _get_programs()
    in_maps, Ps = _prep_inmaps(inputs)
    res1 = run_bass_kernel_spmd(nc1, in_maps, list(range(8))).results

    y_merged = np.zeros((B, Di, L), np.float32)
    sz_full = [None] * B
    x_full = [None] * B
    for core in range(8):
        b, k = core // 4, core % 4
        y_merged[b][:, Ps[k]] += res1[core]["y_out"]
        if k == 0:
            sz_full[b] = res1[core]["sz_out"]
            x_full[b] = res1[core]["x_out"]

    in_maps2 = _prep_inmaps2(inputs, y_merged, sz_full, x_full)
    res2 = run_bass_kernel_spmd(nc2, in_maps2, list(range(8))).results

    out = np.zeros((B, D, L), np.float32)
    for core in range(8):
        b, sl = core // 4, slice((core % 4) * T2, (core % 4 + 1) * T2)
        out[b][:, sl] = res2[core]["out"]
    return out.reshape(B, D, H, W)



# revision 17
# speedup vs baseline: 1.4738x; 1.4738x over previous
"""Trainium2 Bass kernel for nn_Decoder_17489106830107 (VMamba VSSBlock decoder).

Sharding: one (batch, scan-direction) pair per core (B=2 x K=4 = 8 cores).
Host pre-permutes each core's inputs into that core's scan coordinate order,
so all 8 cores run ONE SPMD program for launch 1:
  conv1x1 -> LN stats (mu folded into the in_proj matmul as an extra
  contraction row) -> in_proj -> depthwise conv -> x_proj/dt ->
  softplus (ln(exp(x)+1), one act table) -> per-state exp/scan/gate with
  B/C rows broadcast across partitions by stride-0 DRAM->SBUF DMAs (bf16,
  enabling 2x DVE tensor ops), scans on DVE, part of the dbu multiplies on
  GPSIMD, and the n-accumulation done on the PE as identity matmuls into
  PSUM.
Host then scatter-adds the 4 directional outputs per batch (computing the
out_norm statistics during that merge pass) and an 8-way token-parallel
launch 2 runs the epilogue (out_norm fixup, gating, out_proj, MLP).
"""
import numpy as np
from contextlib import ExitStack

import concourse.bacc as bacc
import concourse.bass as bass
import concourse.mybir as mybir
import concourse.tile as tile
from concourse.bass_utils import run_bass_kernel_spmd
import ml_dtypes

F32 = mybir.dt.float32
F32R = mybir.dt.float32r
BF16 = mybir.dt.bfloat16
AF = mybir.ActivationFunctionType
OP = mybir.AluOpType

B, C, H, W = 2, 256, 64, 64
D = 64
Di = 128
N = 16
R = 4
K = 4
L = H * W          # 4096
LH = L // 2        # half, 2048
LC = 1024          # scan-phase psum chunk
EPS = 1e-5
T2 = 1024          # launch-2 token slice per core

# dbu multiplies sent to GPSIMD (rest on DVE): n in POOL_DBU
POOL_DBU = set(range(14))


def _perms():
    ar = np.arange(L)
    p1 = (ar % 64) * 64 + ar // 64
    return [ar, p1, ar[::-1].copy(), p1[::-1].copy()]


def _permute_kernel(w, k):
    if k == 0:
        return w
    if k == 1:
        return w.transpose(0, 2, 1)
    if k == 2:
        return w[:, ::-1, ::-1]
    return w.transpose(0, 2, 1)[:, ::-1, ::-1]


# ---------------------------------------------------------------- launch 1

def build_launch1():
    nc = bacc.Bacc("TRN2", target_bir_lowering=False, debug=False,
                   num_devices=8)

    xin = nc.dram_tensor("xin", [3 * C, L], BF16, kind="ExternalInput")
    convT = nc.dram_tensor("convT", [3 * C, D], BF16, kind="ExternalInput")
    conv_b = nc.dram_tensor("conv_b", [D, 1], F32, kind="ExternalInput")
    ones64 = nc.dram_tensor("ones64", [D, 1], BF16, kind="ExternalInput")
    Wp_aug = nc.dram_tensor("Wp_aug", [D + 1, 2 * Di], BF16,
                            kind="ExternalInput")
    dwdiag = nc.dram_tensor("dwdiag", [9, 128, 128], BF16,
                            kind="ExternalInput")
    bias_dw = nc.dram_tensor("bias_dw", [Di, 1], F32, kind="ExternalInput")
    xprojT = nc.dram_tensor("xprojT", [Di, R + 2 * N], BF16,
                            kind="ExternalInput")
    dtT = nc.dram_tensor("dtT", [R, Di], BF16, kind="ExternalInput")
    dtb = nc.dram_tensor("dtb", [Di, 1], F32, kind="ExternalInput")
    negA = nc.dram_tensor("negA", [Di, N], F32, kind="ExternalInput")
    Dsdiag = nc.dram_tensor("Dsdiag", [128, 128], BF16, kind="ExternalInput")
    ident = nc.dram_tensor("ident", [128, 128], BF16, kind="ExternalInput")

    x_out = nc.dram_tensor("x_out", [D, L], BF16, kind="ExternalOutput")
    z_out = nc.dram_tensor("z_out", [Di, L], BF16, kind="ExternalOutput")
    y_out = nc.dram_tensor("y_out", [Di, L], BF16, kind="ExternalOutput")

    with tile.TileContext(nc) as tc, ExitStack() as ctx:
        cpool = ctx.enter_context(tc.tile_pool(name="consts", bufs=1))
        main = ctx.enter_context(tc.tile_pool(name="main", bufs=1))
        dpool = ctx.enter_context(tc.tile_pool(name="dram", bufs=1,
                                               space="DRAM"))

        # ---- const loads
        convT_sb = cpool.tile([128, 6, D], BF16, tag="convT")
        nc.sync.dma_start(convT_sb[:],
                          convT[:].rearrange("(c p) m -> p c m", p=128))
        conv_b_sb = cpool.tile([D, 1], F32, tag="convb")
        nc.sync.dma_start(conv_b_sb[:], conv_b[:])
        ones64_sb = cpool.tile([D, 1], BF16, tag="ones64")
        nc.sync.dma_start(ones64_sb[:], ones64[:])
        Wp_sb = cpool.tile([D + 1, 2 * Di], BF16, tag="Wp")
        nc.sync.dma_start(Wp_sb[:], Wp_aug[:])
        dwdiag_sb = cpool.tile([128, 9, 128], BF16, tag="dwdiag")
        nc.sync.dma_start(dwdiag_sb[:], dwdiag[:].rearrange("t p f -> p t f"))
        bias_dw_sb = cpool.tile([Di, 1], F32, tag="biasdw")
        nc.sync.dma_start(bias_dw_sb[:], bias_dw[:])
        xprojT_sb = cpool.tile([Di, R + 2 * N], BF16, tag="xprojT")
        nc.sync.dma_start(xprojT_sb[:], xprojT[:])
        dtT_sb = cpool.tile([R, Di], BF16, tag="dtT")
        nc.sync.dma_start(dtT_sb[:], dtT[:])
        dtb_sb = cpool.tile([Di, 1], F32, tag="dtb")
        nc.sync.dma_start(dtb_sb[:], dtb[:])
        negA_sb = cpool.tile([Di, N], F32, tag="negA")
        nc.sync.dma_start(negA_sb[:], negA[:])
        Dsdiag_sb = cpool.tile([128, 128], BF16, tag="Dsdiag")
        nc.sync.dma_start(Dsdiag_sb[:], Dsdiag[:])
        ident_sb = cpool.tile([128, 128], BF16, tag="ident")
        nc.sync.dma_start(ident_sb[:], ident[:])
        eps_sb = cpool.tile([128, 1], F32, tag="eps")
        nc.vector.memset(eps_sb[:], EPS)

        # persistent activations
        lnin_bf = main.tile([D + 1, L], BF16, tag="lninbf")
        rs_row = main.tile([1, L], F32, tag="rsrow")
        xc_sb = main.tile([Di, L], BF16, tag="xc")
        xdbl_bf = main.tile([R + 2 * N, L], BF16, tag="xdblbf")
        delta_sb = main.tile([Di, L], F32, tag="delta")
        du_sb = main.tile([Di, L], BF16, tag="du")
        # B/C rows staged in DRAM for stride-0 partition-broadcast reads
        xdd = dpool.tile([2 * N, L], BF16, tag="xdd")

        # ---- conv1x1 (psum accumulate over 6 input-channel chunks)
        with tc.tile_pool(name="xinp", bufs=2) as xinp, \
             tc.tile_pool(name="ps_conv", bufs=1, space="PSUM") as ps_conv:
            cps = [ps_conv.tile([D, 512], F32, tag=f"cps{f}", name=f"cps{f}")
                   for f in range(8)]
            for c in range(6):
                xin_c = xinp.tile([128, L], BF16, tag="xin")
                nc.sync.dma_start(xin_c[:], xin[:][c * 128:(c + 1) * 128, :])
                for f in range(8):
                    nc.tensor.matmul(cps[f][:], convT_sb[:, c, :],
                                     xin_c[:, f * 512:(f + 1) * 512],
                                     start=(c == 0), stop=(c == 5))
            for f in range(4):
                fsl = slice(f * 1024, (f + 1) * 1024)
                nc.scalar.activation(lnin_bf[0:D, f * 1024:f * 1024 + 512],
                                     cps[2 * f][:], AF.Identity,
                                     bias=conv_b_sb[:, 0:1])
                nc.scalar.activation(lnin_bf[0:D, f * 1024 + 512:(f + 1) * 1024],
                                     cps[2 * f + 1][:], AF.Identity,
                                     bias=conv_b_sb[:, 0:1])
        nc.sync.dma_start(x_out[:], lnin_bf[0:D, :])

        # ---- LN1 stats per half -> mu row (bf16, folded into in_proj rhs)
        #      and rs row (f32, broadcast below)
        with tc.tile_pool(name="sqp", bufs=1) as sqp:
            sq_bf = sqp.tile([D, L], BF16, tag="sqbf")
            nc.scalar.square(sq_bf[:], lnin_bf[0:D, :])
            for hh in range(2):
                hsl = slice(hh * LH, (hh + 1) * LH)
                with tc.tile_pool(name="ps_st", bufs=1, space="PSUM") as ps_st:
                    st0 = ps_st.tile([1, LH], F32, tag="st0", name="st0")
                    st1 = ps_st.tile([1, LH], F32, tag="st1", name="st1")
                    for f in range(4):
                        fsl = slice(hh * LH + f * 512,
                                    hh * LH + (f + 1) * 512)
                        psl = slice(f * 512, (f + 1) * 512)
                        nc.tensor.matmul(st0[:, psl], ones64_sb[:],
                                         lnin_bf[0:D, fsl], start=True,
                                         stop=True)
                        nc.tensor.matmul(st1[:, psl], ones64_sb[:],
                                         sq_bf[:, fsl], start=True, stop=True)
                    st0_sb = sqp.tile([1, LH], F32, tag="st0sb", bufs=2)
                    st1_sb = sqp.tile([1, LH], F32, tag="st1sb", bufs=2)
                    nc.scalar.copy(st0_sb[:], st0[:])
                    nc.scalar.copy(st1_sb[:], st1[:])
                s0r = sqp.tile([128, 16], F32, tag="s0r", bufs=2)
                s1r = sqp.tile([128, 16], F32, tag="s1r", bufs=2)
                nc.sync.dma_start(s0r[:], st0_sb[:])
                nc.sync.dma_start(s1r[:], st1_sb[:])
                m_r = sqp.tile([128, 16], F32, tag="m_r", bufs=2)
                nc.scalar.mul(m_r[:], s0r[:], 1.0 / D)
                m_bf = sqp.tile([128, 16], BF16, tag="m_bf", bufs=2)
                nc.scalar.copy(m_bf[:], m_r[:])
                msq = sqp.tile([128, 16], F32, tag="msq", bufs=2)
                nc.scalar.square(msq[:], m_r[:])
                v_r = sqp.tile([128, 16], F32, tag="v_r", bufs=2)
                nc.vector.scalar_tensor_tensor(v_r[:], s1r[:], 1.0 / D,
                                               msq[:], OP.mult, OP.subtract)
                sd_r = sqp.tile([128, 16], F32, tag="sd_r", bufs=2)
                nc.scalar.activation(sd_r[:], v_r[:], AF.Sqrt,
                                     bias=eps_sb[:, 0:1])
                rs_r = sqp.tile([128, 16], F32, tag="rs_r", bufs=2)
                nc.vector.reciprocal(rs_r[:], sd_r[:])
                nc.sync.dma_start(lnin_bf[D:D + 1, hsl], m_bf[:])
                nc.sync.dma_start(rs_row[:, hsl], rs_r[:])
            rs_b = main.tile([128, L], F32, tag="rs_b")
            nc.gpsimd.partition_broadcast(rs_b[:], rs_row[:])

        # ---- per-half prep + scan pipeline
        with tc.tile_pool(name="imgp", bufs=1) as imgp:
            img = imgp.tile([Di, 66 * 66], BF16, tag="img")
            nc.gpsimd.memset(img[:], 0.0)
            img3 = img[:].rearrange("p (h w) -> p h w", h=66)

            with tc.tile_pool(name="ps_ip", bufs=1, space="PSUM") as ps_ip, \
                 tc.tile_pool(name="ps_dw", bufs=1, space="PSUM") as ps_dw, \
                 tc.tile_pool(name="ps_xp", bufs=1, space="PSUM") as ps_xp, \
                 tc.tile_pool(name="ps_y", bufs=1, space="PSUM") as ps_y, \
                 tc.tile_pool(name="bc", bufs=4) as bcp, \
                 tc.tile_pool(name="nl", bufs=2) as nl, \
                 tc.tile_pool(name="earlyz", bufs=2) as zp:

                for hh in range(2):
                    # --- in_proj (mu-row folded) + z + img interior
                    for f in range(4):
                        fsl = slice(hh * LH + f * 512, hh * LH + (f + 1) * 512)
                        r0 = (hh * 4 + f) * 8
                        pp = ps_ip.tile([Di, 512], F32, tag="ipps")
                        nc.tensor.matmul(pp[:], Wp_sb[:, 0:Di],
                                         lnin_bf[:, fsl], start=True,
                                         stop=True)
                        nc.vector.tensor_tensor(
                            img3[:, 1 + r0:1 + r0 + 8, 1:65],
                            pp[:].rearrange("p (r w) -> p r w", r=8),
                            rs_b[:, fsl].rearrange("p (r w) -> p r w", r=8),
                            OP.mult)
                        ppz = ps_ip.tile([Di, 512], F32, tag="zps")
                        nc.tensor.matmul(ppz[:], Wp_sb[:, Di:2 * Di],
                                         lnin_bf[:, fsl], start=True,
                                         stop=True)
                        zt = zp.tile([Di, 512], BF16, tag="zt")
                        nc.scalar.copy(zt[:], ppz[:])
                        nc.sync.dma_start(z_out[:][:, fsl], zt[:])

                    # --- dwconv + silu -> xc ; xproj ; dt -> E=exp(dt+b)
                    E_sb = nl.tile([Di, LH], F32, tag="E", bufs=2)
                    for f in range(4):
                        fa = hh * 4 + f
                        fsl = slice(fa * 512, (fa + 1) * 512)
                        r0 = fa * 8
                        dps = ps_dw.tile([128, 512], F32, tag="dwps")
                        for t in range(9):
                            di_, dj = t // 3, t % 3
                            nc.tensor.matmul(
                                dps[:], dwdiag_sb[:, t, :],
                                img3[:, r0 + di_:r0 + di_ + 8, dj:dj + 64],
                                start=(t == 0), stop=(t == 8))
                        nc.scalar.activation(xc_sb[:, fsl], dps[:], AF.Silu,
                                             bias=bias_dw_sb[:, 0:1])
                        xps = ps_xp.tile([R + 2 * N, 512], F32, tag="xpps")
                        nc.tensor.matmul(xps[:], xprojT_sb[:], xc_sb[:, fsl],
                                         start=True, stop=True)
                        nc.scalar.copy(xdbl_bf[:, fsl], xps[:])
                        nc.sync.dma_start(xdd[:][:, fsl],
                                          xdbl_bf[R:R + 2 * N, fsl])
                        dtps = ps_dw.tile([Di, 512], F32, tag="dwps")
                        nc.tensor.matmul(dtps[:], dtT_sb[:],
                                         xdbl_bf[0:R, fsl],
                                         start=True, stop=True)
                        nc.scalar.activation(E_sb[:, f * 512:(f + 1) * 512],
                                             dtps[:], AF.Exp,
                                             bias=dtb_sb[:, 0:1])
                    hsl = slice(hh * LH, (hh + 1) * LH)
                    # delta = softplus = ln(E + 1); du = delta * xc
                    nc.scalar.activation(delta_sb[:, hsl], E_sb[:], AF.Ln,
                                         bias=1.0)
                    nc.vector.tensor_tensor(du_sb[:, hsl], delta_sb[:, hsl],
                                            xc_sb[:, hsl], OP.mult)

                    # --- scan block for this half
                    y_ps = ps_y.tile([Di, LH], F32, tag="yps")
                    for j in range(LH // 512):
                        jsl = slice(j * 512, (j + 1) * 512)
                        nc.tensor.matmul(y_ps[:, jsl], Dsdiag_sb[:],
                                         xc_sb[:, hh * LH + j * 512:
                                               hh * LH + (j + 1) * 512],
                                         start=True, stop=False,
                                         skip_group_check=True)
                    for n in range(N):
                        bb = bcp.tile([128, LH], BF16, tag="bb")
                        nc.sync.dma_start(
                            bb[:],
                            xdd[n:n + 1, hsl].partition_broadcast(128)
                            .squeeze(1))
                        cb = bcp.tile([128, LH], BF16, tag="cb")
                        nc.sync.dma_start(
                            cb[:],
                            xdd[N + n:N + n + 1, hsl].partition_broadcast(128)
                            .squeeze(1))
                        da = nl.tile([Di, LH], F32, tag="da")
                        nc.scalar.activation(da[:], delta_sb[:, hsl], AF.Exp,
                                             scale=negA_sb[:, n:n + 1])
                        dbu = nl.tile([Di, LH], BF16, tag="dbu")
                        if n in POOL_DBU:
                            nc.gpsimd.tensor_tensor(dbu[:], du_sb[:, hsl],
                                                    bb[:], OP.mult)
                        else:
                            nc.vector.tensor_tensor(dbu[:], du_sb[:, hsl],
                                                    bb[:], OP.mult)
                        h = nl.tile([Di, LH], BF16, tag="h", bufs=2)
                        if hh == 0:
                            nc.vector.tensor_tensor_scan(
                                h[:], da[:], dbu[:], 0.0, OP.mult, OP.add)
                        else:
                            nc.vector.tensor_tensor_scan(
                                h[:], da[:], dbu[:], carry[n][:, 0:1],
                                OP.mult, OP.add)
                        if hh == 0:
                            if n == 0:
                                carry = {}
                            carry_t = main.tile([Di, 1], F32,
                                                tag=f"carry{n}",
                                                name=f"carry{n}")
                            carry[n] = carry_t
                            nc.gpsimd.tensor_copy(carry_t[:],
                                                  h[:, LH - 1:LH])
                        tmp = nl.tile([Di, LH], BF16, tag="tmp")
                        nc.vector.tensor_tensor(tmp[:], h[:], cb[:], OP.mult)
                        for j in range(LH // 512):
                            jsl = slice(j * 512, (j + 1) * 512)
                            nc.tensor.matmul(y_ps[:, jsl], ident_sb[:],
                                             tmp[:, jsl],
                                             start=False, stop=(n == N - 1),
                                             skip_group_check=True)
                    y_sb = nl.tile([Di, LH], BF16, tag="ysb", bufs=2)
                    nc.vector.tensor_copy(y_sb[:], y_ps[:])
                    nc.sync.dma_start(y_out[:][:, hsl], y_sb[:])

    nc.compile()
    return nc


# ---------------------------------------------------------------- launch 2

def build_launch2():
    nc = bacc.Bacc("TRN2", target_bir_lowering=False, debug=False,
                   num_devices=8)

    y_in = nc.dram_tensor("y_in", [Di, T2], BF16, kind="ExternalInput")
    z_in = nc.dram_tensor("z_in", [Di, T2], BF16, kind="ExternalInput")
    x_in = nc.dram_tensor("x_in", [D, T2], BF16, kind="ExternalInput")
    rows_in = nc.dram_tensor("rows_in", [3, T2], F32, kind="ExternalInput")
    onorm_g = nc.dram_tensor("onorm_g", [Di, 1], F32, kind="ExternalInput")
    onorm_b = nc.dram_tensor("onorm_b", [Di, 1], F32, kind="ExternalInput")
    bias_z = nc.dram_tensor("bias_z", [Di, 1], F32, kind="ExternalInput")
    oproj = nc.dram_tensor("oproj", [Di, D], BF16, kind="ExternalInput")
    ones64 = nc.dram_tensor("ones64", [D, 1], BF16, kind="ExternalInput")
    fc1p = nc.dram_tensor("fc1p", [D + 1, 2 * Di], BF16,
                          kind="ExternalInput")
    bias1 = nc.dram_tensor("bias1", [128, 2], F32, kind="ExternalInput")
    fc2w = nc.dram_tensor("fc2w", [2 * Di, D], BF16, kind="ExternalInput")
    fc2b = nc.dram_tensor("fc2b", [D, 1], F32, kind="ExternalInput")
    out = nc.dram_tensor("out", [D, T2], F32, kind="ExternalOutput")

    with tile.TileContext(nc) as tc, ExitStack() as ctx:
        po = ctx.enter_context(tc.tile_pool(name="main", bufs=1))

        y_sb = po.tile([Di, T2], BF16, tag="y")
        nc.sync.dma_start(y_sb[:], y_in[:])
        z_sb = po.tile([Di, T2], BF16, tag="z")
        nc.sync.dma_start(z_sb[:], z_in[:])
        x_sb = po.tile([D, T2], BF16, tag="x")
        nc.sync.dma_start(x_sb[:], x_in[:])
        row0_sb = po.tile([1, T2], F32, tag="row0")
        nc.sync.dma_start(row0_sb[:], rows_in[0:1, :])
        row1_sb = po.tile([1, T2], F32, tag="row1")
        nc.sync.dma_start(row1_sb[:], rows_in[1:2, :])
        row2_sb = po.tile([1, T2], F32, tag="row2")
        nc.sync.dma_start(row2_sb[:], rows_in[2:3, :])
        og_sb = po.tile([Di, 1], F32, tag="og")
        nc.sync.dma_start(og_sb[:], onorm_g[:])
        ob_sb = po.tile([Di, 1], F32, tag="ob")
        nc.sync.dma_start(ob_sb[:], onorm_b[:])
        bz_sb = po.tile([Di, 1], F32, tag="bz")
        nc.sync.dma_start(bz_sb[:], bias_z[:])
        op_sb = po.tile([Di, D], BF16, tag="oproj")
        nc.sync.dma_start(op_sb[:], oproj[:])
        ones_sb = po.tile([D, 1], BF16, tag="ones")
        nc.sync.dma_start(ones_sb[:], ones64[:])
        fc1_sb = po.tile([D + 1, 2 * Di], BF16, tag="fc1")
        nc.sync.dma_start(fc1_sb[:], fc1p[:])
        b1_sb = po.tile([128, 2], F32, tag="b1")
        nc.sync.dma_start(b1_sb[:], bias1[:])
        fc2_sb = po.tile([128, 2, D], BF16, tag="fc2")
        nc.sync.dma_start(fc2_sb[:],
                          fc2w[:].rearrange("(c p) m -> p c m", p=128))
        fc2b_sb = po.tile([D, 1], F32, tag="fc2b")
        nc.sync.dma_start(fc2b_sb[:], fc2b[:])
        eps_sb = po.tile([128, 1], F32, tag="eps")
        nc.vector.memset(eps_sb[:], EPS)

        # broadcast host-computed rows: rs1 (for z), mu_y*rs_y, rs_y
        rs1_b = po.tile([Di, T2], F32, tag="rs1b")
        nc.gpsimd.partition_broadcast(rs1_b[:], row0_sb[:])
        murs_b = po.tile([Di, T2], F32, tag="mursb")
        nc.gpsimd.partition_broadcast(murs_b[:], row1_sb[:])
        rsy_b = po.tile([Di, T2], F32, tag="rsyb")
        nc.gpsimd.partition_broadcast(rsy_b[:], row2_sb[:])

        # z gate: sz = silu(z*rs1 + bias_z)
        zr = po.tile([Di, T2], F32, tag="zr")
        nc.vector.tensor_tensor(zr[:], z_sb[:], rs1_b[:], OP.mult)
        sz = po.tile([Di, T2], F32, tag="sz")
        nc.scalar.activation(sz[:], zr[:], AF.Silu, bias=bz_sb[:, 0:1])

        # out_norm fixup: yn = (y*rs_y - mu_y*rs_y)*g + b ; yg = yn*sz
        t1 = po.tile([Di, T2], F32, tag="t1")
        nc.vector.tensor_tensor(t1[:], y_sb[:], rsy_b[:], OP.mult)
        t2 = po.tile([Di, T2], F32, tag="t2")
        nc.vector.tensor_tensor(t2[:], t1[:], murs_b[:], OP.subtract)
        t3 = po.tile([Di, T2], F32, tag="t3")
        nc.vector.tensor_scalar(t3[:], t2[:], og_sb[:, 0:1], ob_sb[:, 0:1],
                                OP.mult, OP.add)
        yg = po.tile([Di, T2], BF16, tag="yg")
        nc.vector.tensor_tensor(yg[:], t3[:], sz[:], OP.mult)

        # out_proj + residual
        x2 = po.tile([D, T2], F32, tag="x2")
        x2sq = po.tile([D, T2], BF16, tag="x2sq")
        with tc.tile_pool(name="ps1", bufs=1, space="PSUM") as ps1:
            opps = ps1.tile([D, T2], F32, tag="opps")
            for f in range(T2 // 512):
                fsl = slice(f * 512, (f + 1) * 512)
                nc.tensor.matmul(opps[:, fsl], op_sb[:], yg[:, fsl],
                                 start=True, stop=True)
            nc.vector.tensor_tensor(x2[:], opps[:], x_sb[:], OP.add)

        # LN2 stats (on-device): mu2 row (bf16, folded into fc1 rhs), rs2
        x2_bf = po.tile([D + 1, T2], BF16, tag="x2bf")
        nc.scalar.copy(x2_bf[0:D, :], x2[:])
        nc.scalar.square(x2sq[:], x2[:])
        with tc.tile_pool(name="ps_st", bufs=1, space="PSUM") as ps_st:
            st0 = ps_st.tile([1, T2], F32, tag="st0")
            st1 = ps_st.tile([1, T2], F32, tag="st1")
            for f in range(T2 // 512):
                fsl = slice(f * 512, (f + 1) * 512)
                nc.tensor.matmul(st0[:, fsl], ones_sb[:],
                                 x2_bf[0:D, fsl], start=True, stop=True)
                nc.tensor.matmul(st1[:, fsl], ones_sb[:],
                                 x2sq[:, fsl], start=True, stop=True)
            st0_sb = po.tile([1, T2], F32, tag="st0sb")
            st1_sb = po.tile([1, T2], F32, tag="st1sb")
            nc.scalar.copy(st0_sb[:], st0[:])
            nc.scalar.copy(st1_sb[:], st1[:])
        s0r = po.tile([128, T2 // 128], F32, tag="s0r")
        s1r = po.tile([128, T2 // 128], F32, tag="s1r")
        nc.sync.dma_start(s0r[:], st0_sb[:])
        nc.sync.dma_start(s1r[:], st1_sb[:])
        m_r = po.tile([128, T2 // 128], F32, tag="m")
        nc.scalar.mul(m_r[:], s0r[:], 1.0 / D)
        m_bf = po.tile([128, T2 // 128], BF16, tag="mbf")
        nc.scalar.copy(m_bf[:], m_r[:])
        msq = po.tile([128, T2 // 128], F32, tag="msq")
        nc.scalar.square(msq[:], m_r[:])
        v_r = po.tile([128, T2 // 128], F32, tag="v")
        nc.vector.scalar_tensor_tensor(v_r[:], s1r[:], 1.0 / D, msq[:],
                                       OP.mult, OP.subtract)
        sd_r = po.tile([128, T2 // 128], F32, tag="sd")
        nc.scalar.activation(sd_r[:], v_r[:], AF.Sqrt, bias=eps_sb[:, 0:1])
        rs_r = po.tile([128, T2 // 128], F32, tag="rs")
        nc.vector.reciprocal(rs_r[:], sd_r[:])
        rs1d = po.tile([1, T2], F32, tag="rs1d")
        nc.sync.dma_start(rs1d[:], rs_r[:])
        nc.sync.dma_start(x2_bf[D:D + 1, :], m_bf[:])
        rs2_b = po.tile([128, T2], F32, tag="rs2b")
        nc.gpsimd.partition_broadcast(rs2_b[:], rs1d[:])

        # fc1 (mu-row folded) -> *rs2 -> gelu -> fc2 -> + residual
        g1 = po.tile([128, 2, T2], BF16, tag="g1")
        with tc.tile_pool(name="ps2", bufs=2, space="PSUM") as ps2:
            for mc in range(2):
                fp = ps2.tile([128, T2], F32, tag="fc1ps")
                for f in range(T2 // 512):
                    fsl = slice(f * 512, (f + 1) * 512)
                    nc.tensor.matmul(fp[:, fsl],
                                     fc1_sb[:, mc * 128:(mc + 1) * 128],
                                     x2_bf[:, fsl], start=True, stop=True)
                fr = po.tile([128, T2], F32, tag="fr", bufs=2)
                nc.vector.tensor_tensor(fr[:], fp[:], rs2_b[:], OP.mult)
                nc.scalar.activation(g1[:, mc, :], fr[:],
                                     AF.Gelu_apprx_tanh,
                                     bias=b1_sb[:, mc:mc + 1])
        with tc.tile_pool(name="ps3", bufs=1, space="PSUM") as ps3:
            f2 = ps3.tile([D, T2], F32, tag="f2ps")
            for f in range(T2 // 512):
                fsl = slice(f * 512, (f + 1) * 512)
                for mc in range(2):
                    nc.tensor.matmul(f2[:, fsl], fc2_sb[:, mc, :],
                                     g1[:, mc, fsl],
                                     start=(mc == 0), stop=(mc == 1))
            o_sb = po.tile([D, T2], F32, tag="o")
            nc.vector.scalar_tensor_tensor(o_sb[:], f2[:],
                                           fc2b_sb[:, 0:1], x2[:],
                                           OP.add, OP.add)
        nc.sync.dma_start(out[:], o_sb[:])

    nc.compile()
    return nc


# ---------------------------------------------------------------- host side

_CACHE = {}


def _get_programs():
    if "nc1" not in _CACHE:
        _CACHE["nc1"] = build_launch1()
        _CACHE["nc2"] = build_launch2()
    return _CACHE["nc1"], _CACHE["nc2"]


def _prep_inmaps(inputs):
    f32 = lambda a: np.ascontiguousarray(np.asarray(a), dtype=np.float32)
    bf = lambda a: np.ascontiguousarray(np.asarray(a, dtype=np.float32)
                                        .astype(ml_dtypes.bfloat16))
    conv_w = f32(inputs["conv_w"])
    conv_b = f32(inputs["conv_b"])
    ln1_g, ln1_b = f32(inputs["ln1_g"]), f32(inputs["ln1_b"])
    in_proj_w = f32(inputs["in_proj_w"])
    dw_w_all = f32(inputs["conv_dw_w"])[:, 0]
    dw_b = f32(inputs["conv_dw_b"])
    x_proj_w = f32(inputs["x_proj_w"])
    dt_proj_w = f32(inputs["dt_proj_w"])
    dt_proj_b = f32(inputs["dt_proj_b"])
    A = np.exp(f32(inputs["A_logs"])).reshape(K, Di, N)
    Ds = f32(inputs["Ds"]).reshape(K, Di)

    Wp = (ln1_g[:, None] * in_proj_w).astype(np.float32)        # [64, 256]
    q = Wp.sum(0)                                                # [256]
    Wp_aug = np.concatenate([Wp, -q[None, :]], 0)                # [65, 256]
    bias_full = (ln1_b @ in_proj_w).astype(np.float32)           # [256]

    Ps = _perms()
    x123 = [np.concatenate([f32(inputs["x1"])[b], f32(inputs["x2"])[b],
                            f32(inputs["x3"])[b]], 0).reshape(3 * C, L)
            for b in range(B)]

    shared = {
        "convT": bf(conv_w.T),
        "conv_b": conv_b.reshape(D, 1).copy(),
        "ones64": np.ones((D, 1), ml_dtypes.bfloat16),
        "Wp_aug": bf(Wp_aug),
        "ident": np.eye(128, dtype=ml_dtypes.bfloat16),
    }
    in_maps = []
    for core in range(8):
        b, k = core // 4, core % 4
        dw_w = _permute_kernel(dw_w_all, k)
        wsum = dw_w.sum((1, 2))
        dwdiag = np.zeros((9, 128, 128), ml_dtypes.bfloat16)
        for t in range(9):
            np.fill_diagonal(dwdiag[t], dw_w[:, t // 3, t % 3])
        in_maps.append({
            **shared,
            "xin": np.ascontiguousarray(
                x123[b][:, Ps[k]].astype(ml_dtypes.bfloat16)),
            "dwdiag": dwdiag,
            "bias_dw": (dw_b + bias_full[:Di] * wsum).reshape(Di, 1)
                        .astype(np.float32),
            "xprojT": bf(x_proj_w[k].T),
            "dtT": bf(dt_proj_w[k].T),
            "dtb": dt_proj_b[k].reshape(Di, 1).astype(np.float32),
            "negA": np.ascontiguousarray(-A[k]),
            "Dsdiag": np.diag(Ds[k]).astype(ml_dtypes.bfloat16),
        })
    return in_maps, Ps, bias_full


def _prep_inmaps2(inputs, y_merged, z_full, x_full, rs1_full, bias_full):
    f32 = lambda a: np.ascontiguousarray(np.asarray(a), dtype=np.float32)
    bf = lambda a: np.ascontiguousarray(np.asarray(a, dtype=np.float32)
                                        .astype(ml_dtypes.bfloat16))
    ln2_g, ln2_b = f32(inputs["ln2_g"]), f32(inputs["ln2_b"])
    fc1_w, fc1_b = f32(inputs["fc1_w"]), f32(inputs["fc1_b"])
    fc1p = (ln2_g[:, None] * fc1_w).astype(np.float32)           # [64, 256]
    q2 = fc1p.sum(0)
    fc1p_aug = np.concatenate([fc1p, -q2[None, :]], 0)           # [65, 256]
    bias1 = (ln2_b @ fc1_w + fc1_b).astype(np.float32)

    # host-side out_norm stats on the merged y (f32 merge pass)
    mu_y = y_merged.mean(1)                                      # [B, L]
    var_y = (y_merged * y_merged).mean(1) - mu_y * mu_y
    rs_y = 1.0 / np.sqrt(var_y + EPS)

    shared = {
        "onorm_g": f32(inputs["out_norm_g"]).reshape(Di, 1).copy(),
        "onorm_b": f32(inputs["out_norm_b"]).reshape(Di, 1).copy(),
        "bias_z": bias_full[Di:].reshape(Di, 1).copy(),
        "oproj": bf(f32(inputs["out_proj_w"])),
        "ones64": np.ones((D, 1), ml_dtypes.bfloat16),
        "fc1p": bf(fc1p_aug),
        "bias1": np.ascontiguousarray(np.stack([bias1[:128], bias1[128:]], 1),
                                      np.float32),
        "fc2w": bf(f32(inputs["fc2_w"])),
        "fc2b": f32(inputs["fc2_b"]).reshape(D, 1).copy(),
    }
    in_maps = []
    for core in range(8):
        b, sl = core // 4, slice((core % 4) * T2, (core % 4 + 1) * T2)
        rows = np.stack([rs1_full[b][sl],
                         (mu_y[b] * rs_y[b])[sl],
                         rs_y[b][sl]], 0).astype(np.float32)
        in_maps.append({
            **shared,
            "y_in": np.ascontiguousarray(y_merged[b][:, sl]
                                         .astype(ml_dtypes.bfloat16)),
            "z_in": np.ascontiguousarray(z_full[b][:, sl]),
            "x_in": np.ascontiguousarray(x_full[b][:, sl]),
            "rows_in": np.ascontiguousarray(rows),
        })
    return in_maps


def kernel(**inputs):
    nc1, nc2 = _get_programs()
    in_maps, Ps, bias_full = _prep_inmaps(inputs)
    res1 = run_bass_kernel_spmd(nc1, in_maps, list(range(8))).results

    y_merged = np.zeros((B, Di, L), np.float32)
    z_full = [None] * B
    x_full = [None] * B
    rs1_full = [None] * B
    for core in range(8):
        b, k = core // 4, core % 4
        y_merged[b][:, Ps[k]] += res1[core]["y_out"].astype(np.float32)
        if k == 0:
            z_full[b] = res1[core]["z_out"]
            x_full[b] = res1[core]["x_out"]
            xf = res1[core]["x_out"].astype(np.float32)
            v1 = (xf * xf).mean(0) - xf.mean(0) ** 2
            rs1_full[b] = 1.0 / np.sqrt(v1 + EPS)

    in_maps2 = _prep_inmaps2(inputs, y_merged, z_full, x_full, rs1_full,
                             bias_full)
    res2 = run_bass_kernel_spmd(nc2, in_maps2, list(range(8))).results

    out = np.zeros((B, D, L), np.float32)
    for core in range(8):
        b, sl = core // 4, slice((core % 4) * T2, (core % 4 + 1) * T2)
        out[b][:, sl] = res2[core]["out"]
    return out.reshape(B, D, H, W)
